# revision 52
# baseline (speedup 1.0000x reference)
"""Trainium2 8-core kernel for nn_Attention_27530740367526.

Multi-head causal attention (B=2, S=2048, D=2048, H=16, HD=128, fp32) with
RoPE, sharded batch x head-group across 8 NeuronCores: core c handles batch
c//4 and heads [4*(c%4), 4*(c%4)+4).  Each core computes q/k/v projections
(+RoPE), attention for its 4 heads, and the slice of the wo projection those
heads feed — a partial [S, D] output.  The host sums the 4 partials per
batch (the row-parallel wo "all-reduce" is a host-side unshard).

Single fused pass: per 512-column sequence chunk (causal order) the kernel
projects q/k/v for all 4 local heads, runs attention for the chunk's queries
(head pairs interleaved so the PE always has two independent softmax chains),
and the previous chunk's wo projection (all 4 heads accumulated in PSUM, one
bf16 output write) drains into the attention's softmax-wait bubbles.

All matmul operands are bf16 (fast weight loads, half the DMA/SBUF), with
fp32 PSUM accumulation; the RoPE rotate-half runs as a f32r 128x128
permutation matmul on the PE.  Scores live in "transposed land" ([k, q] with
head-dim contraction) so softmax denominators come from an all-ones matmul
and PV/wo consume natural layouts with zero on-device transposes.  Diagonal
score tiles are narrowed to skip fully-masked columns.  Every DRAM tensor is
host-pre-tiled so each DMA descriptor is contiguous per partition.
"""

import sys

if "/opt/trn_rl_repo" not in sys.path:
    sys.path.insert(0, "/opt/trn_rl_repo")

from collections import deque

import numpy as np
import ml_dtypes

import concourse.bacc as bacc
import concourse.mybir as mybir
import concourse.tile as tile
from concourse.bass_utils import run_bass_kernel_spmd

F32 = mybir.dt.float32
F32R = mybir.dt.float32r
BF16 = mybir.dt.bfloat16
F16 = mybir.dt.float16
AF = mybir.ActivationFunctionType

N_HEADS = 16
N_CORES = 8
B, S, D = 2, 2048, 2048
HD = D // N_HEADS
H_LOC = N_HEADS // (N_CORES // B)  # 4 heads per core
SC = 512                           # seq chunk (matmul moving free dim)
P = 128
KO = D // P                        # 16 contraction subtiles for projections
NQC = S // SC                      # 4 q-chunks
NSUB = SC // P                     # 4 128-blocks per chunk
NST = S // P                       # 16 s-tiles
QKV_W = 3 * H_LOC * HD             # 1536 packed qkv columns
LOOKAHEAD = 3                      # scores-tile software pipeline depth


def _build_core_kernel():
    inv_sqrt_hd = 1.0 / float(np.sqrt(HD))

    nc = bacc.Bacc(None, target_bir_lowering=False)

    # host-pre-tiled inputs: every slice below is contiguous per partition
    xt_d = nc.dram_tensor("xt", [NQC, P, KO, SC], BF16, kind="ExternalInput")
    wqk_d = nc.dram_tensor(
        "wqk", [KO, P, 2 * H_LOC * HD], BF16, kind="ExternalInput"
    )
    wv_d = nc.dram_tensor(
        "wv", [P, KO, H_LOC * HD], BF16, kind="ExternalInput"
    )
    wo_d = nc.dram_tensor("wo", [P, H_LOC, D], BF16, kind="ExternalInput")
    cs_d = nc.dram_tensor("cs", [NQC, 2, P, SC], F32, kind="ExternalInput")
    pt_d = nc.dram_tensor("pt", [P, HD], F32R, kind="ExternalInput")
    ones_d = nc.dram_tensor("ones", [P, P], BF16, kind="ExternalInput")
    mask_d = nc.dram_tensor("mask", [P, NSUB, P], F32, kind="ExternalInput")
    y = nc.dram_tensor("y", [S, D], BF16, kind="ExternalOutput")

    with tile.TileContext(nc) as tc:
        with (
            tc.tile_pool(name="persist", bufs=1) as persist,
            tc.tile_pool(name="xa", bufs=2) as xa,
            tc.tile_pool(name="cs", bufs=2) as cspool,
            tc.tile_pool(name="scr", bufs=2) as scr,
            tc.tile_pool(name="exps", bufs=4) as expp,
            tc.tile_pool(name="outq", bufs=2) as outqp,
            tc.tile_pool(name="yo", bufs=4) as yop,
            tc.tile_pool(name="accp", bufs=2) as accp,
            tc.tile_pool(name="ps", bufs=3, space="PSUM") as cyc,
            tc.tile_pool(name="ops", bufs=3, space="PSUM") as ops,
            tc.tile_pool(name="yps", bufs=2, space="PSUM") as yps,
        ):
            # small persistent constants (scalar queue, ahead of big loads)
            pt_sb = persist.tile([P, HD], F32R)
            nc.scalar.dma_start(pt_sb[:], pt_d[:])
            ones_sb = persist.tile([P, P], BF16)
            nc.scalar.dma_start(ones_sb[:], ones_d[:])
            mask_sb = persist.tile([P, NSUB, P], F32)
            nc.scalar.dma_start(mask_sb[:], mask_d[:])
            expb = persist.tile([P, 1], F32)
            nc.gpsimd.memset(expb[:], -2.772588722239781)  # -ln(16)

            # per-head-pair persistent k/v for the whole sequence
            kT_sb = persist.tile([P, H_LOC, S], BF16)
            v_sb = persist.tile([P, NST, H_LOC * HD], BF16)
            wqk_sb = persist.tile([P, KO, 2 * H_LOC * HD], BF16)
            wv_sb = persist.tile([P, KO, H_LOC * HD], BF16)
            wo_sb = persist.tile([P, H_LOC, D], BF16)

            def load_chunk(sc):
                # prefetched a chunk ahead: one fully-contiguous descriptor
                # on the scalar queue, which is idle of DMA work by then
                xt = xa.tile([P, KO, SC], BF16, tag="xt")
                nc.scalar.dma_start(xt[:], xt_d[sc])
                cos_t = cspool.tile([P, SC], F32, tag="cos")
                sin_t = cspool.tile([P, SC], F32, tag="sin")
                nc.scalar.dma_start(cos_t[:], cs_d[sc, 0])
                nc.scalar.dma_start(sin_t[:], cs_d[sc, 1])
                return xt, cos_t, sin_t

            # startup: both DMA rings stripe over the same 16 engines, so
            # feed the startup-critical stream alone and in consumption
            # order — (xt0[ko], wv[ko]) pairs feed the chunk-0 v
            # projections (which run first for chunk 0), wqk streams in
            # behind for the q/k groups; wo is deferred to the scalar
            # queue's post-projection emission point
            xt0 = xa.tile([P, KO, SC], BF16, tag="xt")
            for kg in range(4):
                nc.sync.dma_start(
                    xt0[:, 4 * kg : 4 * kg + 4],
                    xt_d[0, :, 4 * kg : 4 * kg + 4],
                )
                nc.sync.dma_start(
                    wv_sb[:, 4 * kg : 4 * kg + 4],
                    wv_d[:, 4 * kg : 4 * kg + 4],
                )
            cos_0 = cspool.tile([P, SC], F32, tag="cos")
            sin_0 = cspool.tile([P, SC], F32, tag="sin")
            nc.sync.dma_start(cos_0[:], cs_d[0, 0])
            nc.sync.dma_start(sin_0[:], cs_d[0, 1])
            preloaded = (xt0, cos_0, sin_0)
            for ko in range(KO):
                nc.sync.dma_start(wqk_sb[:, ko], wqk_d[ko])

            # dummy matmuls: trip the PE HAM clock-gate to full rate while
            # the first (xt0, wv) slices stream in.  They write the
            # (startup-idle) y PSUM bank; the bufs=1 WAW chain serializes
            # them on the PE only.
            def warmup(n):
                for wu in range(n):
                    wps = yps.tile([P, SC], F32, tag="y")
                    nc.tensor.matmul(
                        wps[:, :P], ones_sb[:], ones_sb[:],
                        skip_group_check=True,
                    )

            warmup(16)

            def project_chunk(sc, loaded):
                ssl = slice(sc * SC, (sc + 1) * SC)
                xt, cos_t, sin_t = loaded
                qT_c = outqp.tile([P, H_LOC, SC], BF16, tag="qTc")

                def qkproj():
                    for h in range(H_LOC):
                        for t in range(2):  # 0=q, 1=k
                            wcols = slice(
                                (2 * h + t) * HD, (2 * h + t + 1) * HD
                            )
                            ps = cyc.tile([P, SC], F32, tag="ps")
                            for ko in range(KO):
                                nc.tensor.matmul(
                                    ps[:],
                                    wqk_sb[:, ko, wcols],
                                    xt[:, ko],
                                    start=(ko == 0),
                                    stop=(ko == KO - 1),
                                )
                            plain = scr.tile([P, SC], F32R, tag="plain")
                            nc.scalar.copy(plain[:], ps[:])
                            rot = cyc.tile([P, SC], F32, tag="ps")
                            nc.tensor.matmul(rot[:], pt_sb[:], plain[:])
                            dst = (
                                qT_c[:, h, :] if t == 0 else kT_sb[:, h, ssl]
                            )
                            # rope: dst = plain*cos + rot*sin
                            pc = scr.tile([P, SC], F32, tag="pc")
                            nc.gpsimd.tensor_mul(pc[:], plain[:], cos_t[:])
                            tmp2 = scr.tile([P, SC], F32, tag="tmp2")
                            nc.vector.tensor_mul(tmp2[:], rot[:], sin_t[:])
                            nc.vector.tensor_add(dst, pc[:], tmp2[:])

                def take_plain(src_ps, on_dve=False, g=0):
                    plain = scr.tile(
                        [P, SC], F32R, tag=f"plain{g % 4}", name="plain"
                    )
                    if on_dve:
                        nc.vector.tensor_copy(plain[:], src_ps[:])
                    else:
                        nc.scalar.copy(plain[:], src_ps[:])
                    return plain

                def rope_rest(h, t, plain):
                    rot = cyc.tile([P, SC], F32, tag="ps")
                    nc.tensor.matmul(rot[:], pt_sb[:], plain[:])
                    dst = qT_c[:, h, :] if t == 0 else kT_sb[:, h, ssl]
                    pc = scr.tile([P, SC], F32, tag="pc")
                    nc.gpsimd.tensor_mul(pc[:], plain[:], cos_t[:])
                    tmp2 = scr.tile([P, SC], F32, tag="tmp2")
                    nc.vector.tensor_mul(tmp2[:], rot[:], sin_t[:])
                    nc.vector.tensor_add(dst, pc[:], tmp2[:])

                def rope_tail(h, t, src_ps):
                    rope_rest(h, t, take_plain(src_ps))

                def chunk0_proj():
                    # ko-major so the PE consumes (xt, wv), then wqk slices
                    # in DMA-arrival order, across borrowed idle PSUM banks
                    vbank = [
                        cyc.tile([P, H_LOC * HD], F32, tag="ps",
                                 name=f"vbank{i}")
                        for i in range(3)
                    ]
                    vbank.append(
                        ops.tile([P, SC], F32, tag="o", name="vbank3")
                    )
                    for ko in range(KO):
                        for sti in range(NSUB):
                            lsl = slice(sti * P, (sti + 1) * P)
                            nc.tensor.matmul(
                                vbank[sti][:, : H_LOC * HD],
                                xt[:, ko, lsl],
                                wv_sb[:, ko],
                                start=(ko == 0),
                                stop=(ko == KO - 1),
                                skip_group_check=True,
                            )
                    for sti in range(NSUB):
                        if sti % 2 == 0:
                            nc.scalar.copy(
                                v_sb[:, sti, :], vbank[sti][:, : H_LOC * HD]
                            )
                        else:
                            nc.vector.tensor_copy(
                                v_sb[:, sti, :], vbank[sti][:, : H_LOC * HD]
                            )
                    qbank = (
                        [cyc.tile([P, SC], F32, tag="ps", name=f"qbank{i}")
                         for i in range(3)]
                        + [ops.tile([P, SC], F32, tag="o", name=f"qbank{3+i}")
                           for i in range(2)]
                        + [yps.tile([P, SC], F32, tag="y", name=f"qbank{5+i}")
                           for i in range(2)]
                    )
                    groups = [(h, t) for h in range(H_LOC) for t in range(2)]
                    for ko in range(KO):
                        for g, (h, t) in enumerate(groups):
                            if g == 7:
                                continue  # bank-limited: group 7 runs after
                            wcols = slice(
                                (2 * h + t) * HD, (2 * h + t + 1) * HD
                            )
                            nc.tensor.matmul(
                                qbank[g][:],
                                wqk_sb[:, ko, wcols],
                                xt[:, ko],
                                start=(ko == 0),
                                stop=(ko == KO - 1),
                                skip_group_check=True,
                            )
                    # free the borrowed banks first (copies split across
                    # ACT+DVE), with group 7's group-major matmuls emitted
                    # in between so the PE stays busy during the copies
                    plains = {}
                    for g in range(2):
                        plains[g] = take_plain(qbank[g], g % 2 == 1, g)
                    h7, t7 = groups[7]
                    ps = ops.tile([P, SC], F32, tag="o", name="ps_g7")
                    wcols = slice((2 * h7 + t7) * HD, (2 * h7 + t7 + 1) * HD)
                    for ko in range(KO):
                        nc.tensor.matmul(
                            ps[:],
                            wqk_sb[:, ko, wcols],
                            xt[:, ko],
                            start=(ko == 0),
                            stop=(ko == KO - 1),
                        )
                    for g in range(2, 7):
                        plains[g] = take_plain(qbank[g], g % 2 == 1, g)
                    for g, (h, t) in enumerate(groups[:7]):
                        rope_rest(h, t, plains[g])
                    rope_tail(h7, t7, ps)

                if sc == 0:
                    # chunk 0 is fed by the startup stream in (xt, wv),
                    # then wqk order — consume in that order
                    chunk0_proj()
                else:
                    # this chunk's v projection already ran as attend
                    # fillers during the previous chunk's attends
                    qkproj()
                return qT_c

            def attend_pair(qc, qT_c, hp, outT_qc, fillers, drain_sched=None):
                """Attention for query chunk qc, heads (2hp, 2hp+1)
                interleaved per k-block (so the PE always has two
                independent softmax chains in flight), writing normalized
                outT [hd, q] slices.  `fillers` is a deque of independent
                PE-work closures drained into the pipeline's tail bubbles.
                Diagonal k-blocks are narrowed to their live q columns."""
                nkb = (qc + 1) * NSUB
                qt = {}
                o_ps = {}
                acc = {}
                for hl in range(2):
                    h = 2 * hp + hl
                    qt[hl] = qT_c[:, h, :]
                    o_ps[hl] = ops.tile([P, SC], F32, tag="o", name=f"o_ps{hl}")
                    acc[hl] = accp.tile(
                        [P, SC], F16, tag=f"acc{hl}", name=f"acc{hl}"
                    )
                stile = {}

                def q0(kb):
                    # first live q column for k-block kb (causal narrowing)
                    j = kb - qc * NSUB
                    return j * P if j > 0 else 0

                def emit_scores(kb, hl):
                    h = 2 * hp + hl
                    c0 = q0(kb)
                    t_ = cyc.tile([P, SC], F32, tag="ps")
                    nc.tensor.matmul(
                        t_[:, c0:],
                        kT_sb[:, h, kb * P : (kb + 1) * P],
                        qt[hl][:, c0:],
                        skip_group_check=True,
                    )
                    j = kb - qc * NSUB
                    if j >= 0:
                        # triangular boundary block only
                        nc.vector.tensor_add(
                            t_[:, c0 : c0 + P],
                            t_[:, c0 : c0 + P],
                            mask_sb[:, j, :],
                        )
                    stile[(kb, hl)] = t_

                if drain_sched is None:
                    drain_sched = lambda i: 1 if i % 2 == 1 else 0
                seq = [(kb, hl) for kb in range(nkb) for hl in range(2)]
                for kb, hl in seq[:LOOKAHEAD]:
                    emit_scores(kb, hl)
                for i, (kb, hl) in enumerate(seq):
                    c0 = q0(kb)
                    h = 2 * hp + hl
                    e = expp.tile([P, SC], F16, tag="e")
                    # bias -ln(16) scales e (and thus both the PV numerator
                    # and the denominator — ratio exact) by 1/16, keeping
                    # the fp16 softmax accumulator far from overflow
                    nc.scalar.activation(
                        e[:, c0:], stile.pop((kb, hl))[:, c0:], AF.Exp,
                        scale=inv_sqrt_hd, bias=expb[:],
                    )
                    nc.tensor.matmul(
                        o_ps[hl][:, c0:],
                        v_sb[:, kb, h * HD : (h + 1) * HD],
                        e[:, c0:],
                        start=(kb == 0),
                        stop=(kb == nkb - 1),
                        skip_group_check=True,
                    )
                    # softmax denominator: accumulate e on the DVE (off the
                    # critical path), partition-broadcast later via one
                    # ones-matmul per head
                    if kb == 0:
                        nc.vector.tensor_copy(acc[hl][:], e[:])
                    else:
                        nc.vector.tensor_add(
                            acc[hl][:, c0:], acc[hl][:, c0:], e[:, c0:]
                        )
                    if i + LOOKAHEAD < len(seq):
                        emit_scores(*seq[i + LOOKAHEAD])
                        for _ in range(drain_sched(i)):
                            if fillers:
                                fillers.popleft()()
                    elif fillers:
                        fillers.popleft()()
                for hl in range(2):
                    h = 2 * hp + hl
                    d_ps = cyc.tile([P, SC], F32, tag="ps")
                    nc.tensor.matmul(
                        d_ps[:], ones_sb[:], acc[hl][:],
                        skip_group_check=True,
                    )
                    recip = scr.tile([P, SC], F32, tag="recip")
                    nc.vector.reciprocal_approx_fast(recip[:], d_ps[:])
                    nc.vector.tensor_mul(
                        outT_qc[:, h, :], o_ps[hl][:], recip[:]
                    )

            def make_out_fillers(qc, outT_qc, tail=False):
                """One closure per (s-tile, d-chunk) block of the wo
                projection for query chunk qc: 4 accumulating matmuls (all
                local heads), a PSUM->SBUF bf16 copy, and the output DMA.
                Tail blocks (after the last attend) ping-pong across the
                now-idle attention PSUM banks and both copy engines."""
                work = []
                for sti in range(NSUB):
                    st = qc * NSUB + sti
                    stsl = slice(sti * P, (sti + 1) * P)
                    for dc in range(D // SC):
                        dsl = slice(dc * SC, (dc + 1) * SC)
                        bi = sti * (D // SC) + dc

                        def blk(st=st, stsl=stsl, dsl=dsl, bi=bi):
                            if tail:
                                pool, tag = [(yps, "y"), (ops, "o")][bi % 2]
                                y_ps = pool.tile([P, SC], F32, tag=tag)
                            else:
                                y_ps = yps.tile([P, SC], F32, tag="y")
                            for h in range(H_LOC):
                                nc.tensor.matmul(
                                    y_ps[:],
                                    outT_qc[:, h, stsl],
                                    wo_sb[:, h, dsl],
                                    start=(h == 0),
                                    stop=(h == H_LOC - 1),
                                )
                            y_sb = yop.tile([P, SC], BF16, tag="ysb")
                            # PSUM->bf16 staging copy alternates DVE/ACT
                            # (gpsimd cannot read PSUM) to keep the DVE,
                            # which carries the softmax accumulation, free
                            if bi % 3 != 2:
                                nc.vector.tensor_copy(y_sb[:], y_ps[:])
                            else:
                                nc.scalar.copy(y_sb[:], y_ps[:])
                            yq = (
                                nc.scalar if (tail and bi % 2 == 1)
                                else nc.sync
                            )
                            yq.dma_start(
                                y[st * P : (st + 1) * P, dsl], y_sb[:]
                            )

                        work.append(blk)
                return work

            def warm_filler():
                wps = yps.tile([P, SC], F32, tag="y")
                nc.tensor.matmul(
                    wps[:, :P], ones_sb[:], ones_sb[:],
                    skip_group_check=True,
                )

            def make_v_units(sc, xt):
                """The v projection of chunk sc as 4 self-contained filler
                units (16 accumulating matmuls + a PSUM->bf16 copy each),
                drained into the previous chunk's attend bubbles."""
                units = []
                for sti in range(NSUB):
                    st = sc * NSUB + sti
                    lsl = slice(sti * P, (sti + 1) * P)

                    def vu(st=st, lsl=lsl, sti=sti):
                        psv = yps.tile([P, SC], F32, tag="y")
                        for ko in range(KO):
                            nc.tensor.matmul(
                                psv[:],
                                xt[:, ko, lsl],
                                wv_sb[:, ko],
                                start=(ko == 0),
                                stop=(ko == KO - 1),
                                skip_group_check=True,
                            )
                        if sti % 2 == 0:
                            nc.scalar.copy(v_sb[:, st, :], psv[:])
                        else:
                            nc.vector.tensor_copy(v_sb[:, st, :], psv[:])

                    vu.kind = "v"
                    units.append(vu)
                return units

            pending = deque()
            chunks = {0: preloaded}
            for sc in range(NQC):
                if 1 <= sc < NQC - 1 and sc + 1 not in chunks:
                    # prefetch next chunk before this chunk's projection
                    chunks[sc + 1] = load_chunk(sc + 1)
                qT_c = project_chunk(sc, chunks[sc])
                if sc == 0:
                    # wo and the chunk-1 prefetch land on the scalar queue
                    # only after chunk 0's projection copies — keeps the
                    # startup stream alone on the shared DMA engines
                    for h in range(H_LOC):
                        nc.scalar.dma_start(wo_sb[:, h], wo_d[:, h])
                    chunks[1] = load_chunk(1)
                # leftover v-units for this chunk must run before attends
                while pending and getattr(pending[0], "kind", None) == "v":
                    pending.popleft()()
                if sc + 1 < NQC:
                    pending.extend(make_v_units(sc + 1, chunks[sc + 1][0]))
                if sc == 0:
                    # qc0 has few fillers yet: drain clock-gate warmers
                    # into its softmax-wait bubbles too
                    pending.extend([warm_filler] * 16)
                outT_qc = outqp.tile([P, H_LOC, SC], BF16, tag="outq")
                ds = (lambda i: 2) if sc == 0 else None
                for hp in range(2):
                    attend_pair(sc, qT_c, hp, outT_qc, pending, ds)
                pending = deque(f for f in pending if f is not warm_filler)
                pending.extend(
                    make_out_fillers(sc, outT_qc, tail=(sc == NQC - 1))
                )
            while pending:
                pending.popleft()()

    nc.compile()
    return nc


_NC_CACHE = {}


def _get_nc():
    if "nc" not in _NC_CACHE:
        _NC_CACHE["nc"] = _build_core_kernel()
    return _NC_CACHE["nc"]


def _rope_perm_T() -> np.ndarray:
    # rotate_half as a matrix: (P_rh @ q)[d] = -q[d+HD/2] for d < HD/2,
    # q[d-HD/2] otherwise.  Returns P_rh.T for use as matmul lhsT.
    P_rh = np.zeros((HD, HD), dtype=np.float32)
    half = HD // 2
    for i in range(half):
        P_rh[i, half + i] = -1.0
        P_rh[half + i, i] = 1.0
    return np.ascontiguousarray(P_rh.T)


def _is_causal(m: np.ndarray) -> bool:
    tril = np.tril(np.ones((S, S), dtype=bool))
    if not np.all(m[tril] == 0.0):
        return False
    upper = m[~tril]
    return bool(upper.size == 0 or np.all(upper <= -1.0e8))


def _bf16(x: np.ndarray) -> np.ndarray:
    return np.ascontiguousarray(x).astype(ml_dtypes.bfloat16)


def _reference_numpy(x, cos, sin, mask, wq, wk, wv, wo):
    # generic-mask fallback (never hit for the causal reference mask)
    def rot_half(t):
        t1, t2 = np.split(t, 2, axis=-1)
        return np.concatenate((-t2, t1), axis=-1)

    H = N_HEADS
    q = (x @ wq.T).reshape(B, S, H, HD).transpose(0, 2, 1, 3)
    k = (x @ wk.T).reshape(B, S, H, HD).transpose(0, 2, 1, 3)
    v = (x @ wv.T).reshape(B, S, H, HD).transpose(0, 2, 1, 3)
    c = cos[None, None]
    s = sin[None, None]
    q = q * c + rot_half(q) * s
    k = k * c + rot_half(k) * s
    scores = np.einsum("bhqd,bhkd->bhqk", q, k) / np.sqrt(np.float32(HD))
    scores = scores + mask
    scores -= scores.max(axis=-1, keepdims=True)
    p = np.exp(scores)
    p /= p.sum(axis=-1, keepdims=True)
    out = np.einsum("bhqk,bhkd->bhqd", p, v)
    out = out.transpose(0, 2, 1, 3).reshape(B, S, D)
    return (out @ wo.T).astype(np.float32)


# module-level: results of the last traced run (for test harnesses)
last_exec_time_ns = None
last_profile_json = None


def kernel(x, cos, sin, mask, wq, wk, wv, wo, _trace=False):
    x = np.asarray(x, dtype=np.float32)
    cos = np.asarray(cos, dtype=np.float32)
    sin = np.asarray(sin, dtype=np.float32)
    mask = np.asarray(mask, dtype=np.float32)
    wq = np.asarray(wq, dtype=np.float32)
    wk = np.asarray(wk, dtype=np.float32)
    wv = np.asarray(wv, dtype=np.float32)
    wo = np.asarray(wo, dtype=np.float32)

    m2d = mask.reshape(S, S)
    if not _is_causal(m2d):
        return _reference_numpy(x, cos, sin, mask, wq, wk, wv, wo)
    nc = _get_nc()

    scale = np.float32(np.sqrt(HD))
    # [k, q] transposed causal boundary blocks: mask_h[ki, j, q_local]
    mt = np.ascontiguousarray((m2d[:SC, :SC] * scale).T).reshape(NSUB, P, NSUB, P)
    mask_h = np.ascontiguousarray(
        np.stack([mt[j, :, j, :] for j in range(NSUB)], axis=1)
    )
    # cos/sin chunk-tiled: cs[sc, {cos,sin}, hd, s_local]
    cs = np.stack([cos.T, sin.T], axis=0).reshape(2, HD, NQC, SC)
    cs = np.ascontiguousarray(cs.transpose(2, 0, 1, 3), dtype=np.float32)
    ptT = _rope_perm_T()
    ones = np.ones((P, P), dtype=np.float32)

    # x chunk-tiled: xt[sc, ki, ko, s_local]
    xts = []
    for b in range(B):
        xT = x[b].T.reshape(KO, P, NQC, SC)
        xts.append(_bf16(xT.transpose(2, 1, 0, 3)))

    in_maps = []
    for c in range(N_CORES):
        b = c // (N_CORES // B)
        hg = c % (N_CORES // B)
        # qkv packed per head: [q_h0|k_h0|...|q_h3|k_h3|v_h0..v_h3],
        # laid out [ko, ki, col]
        cols = []
        for h in range(H_LOC):
            hh = hg * H_LOC + h
            cols.append(wq[hh * HD : (hh + 1) * HD].T)
            cols.append(wk[hh * HD : (hh + 1) * HD].T)
        for h in range(H_LOC):
            hh = hg * H_LOC + h
            cols.append(wv[hh * HD : (hh + 1) * HD].T)
        wpack = np.concatenate(cols, axis=1)  # [D, 1536]
        wpack = wpack.reshape(KO, P, QKV_W)
        wqk_h = np.ascontiguousarray(wpack[:, :, : 2 * H_LOC * HD])
        wv_h = np.ascontiguousarray(
            wpack[:, :, 2 * H_LOC * HD :].transpose(1, 0, 2)
        )
        # wo rows for this head group, laid out [ki, h, d]
        rows = slice(hg * H_LOC * HD, (hg + 1) * H_LOC * HD)
        wot = wo[:, rows].T.reshape(H_LOC, P, D)
        wot = np.ascontiguousarray(wot.transpose(1, 0, 2))
        in_maps.append(
            {
                "xt": xts[b],
                "wqk": _bf16(wqk_h),
                "wv": _bf16(wv_h),
                "wo": _bf16(wot),
                "cs": cs,
                "pt": ptT,
                "ones": _bf16(ones),
                "mask": mask_h,
            }
        )

    kw = {}
    if _trace:
        kw = dict(trace=True)
    res = run_bass_kernel_spmd(
        nc, in_maps, core_ids=list(range(N_CORES)), **kw
    )
    global last_exec_time_ns, last_profile_json
    last_exec_time_ns = res.exec_time_ns
    last_profile_json = res.profile_json

    out = np.empty((B, S, D), dtype=np.float32)
    gs = N_CORES // B
    for b in range(B):
        acc = res.results[b * gs]["y"].astype(np.float32)
        for g in range(1, gs):
            acc += res.results[b * gs + g]["y"].astype(np.float32)
        out[b] = acc
    return out


# revision 55
# speedup vs baseline: 1.1210x; 1.1210x over previous
"""Trainium2 8-core kernel for nn_Attention_27530740367526.

Multi-head causal attention (B=2, S=2048, D=2048, H=16, HD=128, fp32) with
RoPE, sharded batch x head-group across 8 NeuronCores: core c handles batch
c//4 and heads [4*(c%4), 4*(c%4)+4).  Each core computes q/k/v projections
(+RoPE), attention for its 4 heads, and the slice of the wo projection those
heads feed — a partial [S, D] output.  The host sums the 4 partials per
batch (the row-parallel wo "all-reduce" is a host-side unshard).

Single fused pass: per 512-column sequence chunk (causal order) the kernel
projects q/k/v for all 4 local heads, runs attention for the chunk's queries
(head pairs interleaved so the PE always has two independent softmax chains),
and the previous chunk's wo projection (all 4 heads accumulated in PSUM, one
bf16 output write) drains into the attention's softmax-wait bubbles.

All matmul operands are bf16 (fast weight loads, half the DMA/SBUF), with
fp32 PSUM accumulation; the RoPE rotate-half runs as a f32r 128x128
permutation matmul on the PE.  Scores live in "transposed land" ([k, q] with
head-dim contraction) so softmax denominators come from an all-ones matmul
and PV/wo consume natural layouts with zero on-device transposes.  Diagonal
score tiles are narrowed to skip fully-masked columns.  Every DRAM tensor is
host-pre-tiled so each DMA descriptor is contiguous per partition.
"""

import sys

if "/opt/trn_rl_repo" not in sys.path:
    sys.path.insert(0, "/opt/trn_rl_repo")

from collections import deque

import numpy as np
import ml_dtypes

import concourse.bacc as bacc
import concourse.mybir as mybir
import concourse.tile as tile
from concourse.bass_utils import run_bass_kernel_spmd

F32 = mybir.dt.float32
F32R = mybir.dt.float32r
BF16 = mybir.dt.bfloat16
F16 = mybir.dt.float16
AF = mybir.ActivationFunctionType

N_HEADS = 16
N_CORES = 8
B, S, D = 2, 2048, 2048
HD = D // N_HEADS
H_LOC = N_HEADS // (N_CORES // B)  # 4 heads per core
SC = 512                           # seq chunk (matmul moving free dim)
P = 128
KO = D // P                        # 16 contraction subtiles for projections
NQC = S // SC                      # 4 q-chunks
NSUB = SC // P                     # 4 128-blocks per chunk
NST = S // P                       # 16 s-tiles
QKV_W = 3 * H_LOC * HD             # 1536 packed qkv columns
LOOKAHEAD = 3                      # scores-tile software pipeline depth


def _build_core_kernel():
    inv_sqrt_hd = 1.0 / float(np.sqrt(HD))

    nc = bacc.Bacc(None, target_bir_lowering=False)

    # host-pre-tiled inputs: every slice below is contiguous per partition
    xt_d = nc.dram_tensor("xt", [NQC, P, KO, SC], BF16, kind="ExternalInput")
    wqk_d = nc.dram_tensor(
        "wqk", [KO, P, 2 * H_LOC * HD], BF16, kind="ExternalInput"
    )
    wv_d = nc.dram_tensor(
        "wv", [P, KO, H_LOC * HD], BF16, kind="ExternalInput"
    )
    wo_d = nc.dram_tensor("wo", [P, H_LOC, D], BF16, kind="ExternalInput")
    cs_d = nc.dram_tensor("cs", [NQC, 2, P, SC], F32, kind="ExternalInput")
    pt_d = nc.dram_tensor("pt", [P, HD], F32R, kind="ExternalInput")
    ones_d = nc.dram_tensor("ones", [P, P], BF16, kind="ExternalInput")
    mask_d = nc.dram_tensor("mask", [P, NSUB, P], F32, kind="ExternalInput")
    y = nc.dram_tensor("y", [S, D], BF16, kind="ExternalOutput")

    with tile.TileContext(nc) as tc:
        with (
            tc.tile_pool(name="persist", bufs=1) as persist,
            tc.tile_pool(name="xa", bufs=2) as xa,
            tc.tile_pool(name="cs", bufs=2) as cspool,
            tc.tile_pool(name="scr", bufs=2) as scr,
            tc.tile_pool(name="exps", bufs=4) as expp,
            tc.tile_pool(name="outq", bufs=2) as outqp,
            tc.tile_pool(name="yo", bufs=4) as yop,
            tc.tile_pool(name="accp", bufs=2) as accp,
            tc.tile_pool(name="ps", bufs=3, space="PSUM") as cyc,
            tc.tile_pool(name="ops", bufs=3, space="PSUM") as ops,
            tc.tile_pool(name="yps", bufs=2, space="PSUM") as yps,
        ):
            # small persistent constants (scalar queue, ahead of big loads)
            pt_sb = persist.tile([P, HD], F32R)
            nc.scalar.dma_start(pt_sb[:], pt_d[:])
            ones_sb = persist.tile([P, P], BF16)
            nc.scalar.dma_start(ones_sb[:], ones_d[:])
            mask_sb = persist.tile([P, NSUB, P], F32)
            nc.scalar.dma_start(mask_sb[:], mask_d[:])
            expb = persist.tile([P, 1], F32)
            nc.gpsimd.memset(expb[:], -2.772588722239781)  # -ln(16)

            # per-head-pair persistent k/v for the whole sequence
            kT_sb = persist.tile([P, H_LOC, S], BF16)
            v_sb = persist.tile([P, NST, H_LOC * HD], BF16)
            wqk_sb = persist.tile([P, KO, 2 * H_LOC * HD], BF16)
            wv_sb = persist.tile([P, KO, H_LOC * HD], BF16)
            wo_sb = persist.tile([P, H_LOC, D], BF16)

            def load_chunk(sc):
                # prefetched a chunk ahead: one fully-contiguous descriptor
                # on the scalar queue, which is idle of DMA work by then
                xt = xa.tile([P, KO, SC], BF16, tag="xt")
                nc.scalar.dma_start(xt[:], xt_d[sc])
                cos_t = cspool.tile([P, SC], F32, tag="cos")
                sin_t = cspool.tile([P, SC], F32, tag="sin")
                nc.scalar.dma_start(cos_t[:], cs_d[sc, 0])
                nc.scalar.dma_start(sin_t[:], cs_d[sc, 1])
                return xt, cos_t, sin_t

            # startup: both DMA rings stripe over the same 16 engines, so
            # feed the startup-critical stream alone and in consumption
            # order — (xt0[ko], wv[ko]) pairs feed the chunk-0 v
            # projections (which run first for chunk 0), wqk streams in
            # behind for the q/k groups; wo is deferred to the scalar
            # queue's post-projection emission point
            xt0 = xa.tile([P, KO, SC], BF16, tag="xt")
            for kg in range(4):
                nc.sync.dma_start(
                    xt0[:, 4 * kg : 4 * kg + 4],
                    xt_d[0, :, 4 * kg : 4 * kg + 4],
                )
                nc.sync.dma_start(
                    wv_sb[:, 4 * kg : 4 * kg + 4],
                    wv_d[:, 4 * kg : 4 * kg + 4],
                )
            cos_0 = cspool.tile([P, SC], F32, tag="cos")
            sin_0 = cspool.tile([P, SC], F32, tag="sin")
            nc.sync.dma_start(cos_0[:], cs_d[0, 0])
            nc.sync.dma_start(sin_0[:], cs_d[0, 1])
            preloaded = (xt0, cos_0, sin_0)
            for ko in range(KO):
                nc.sync.dma_start(wqk_sb[:, ko], wqk_d[ko])

            # dummy matmuls: trip the PE HAM clock-gate to full rate while
            # the first (xt0, wv) slices stream in.  They write the
            # (startup-idle) y PSUM bank; the bufs=1 WAW chain serializes
            # them on the PE only.
            def warmup(n):
                for wu in range(n):
                    wps = yps.tile([P, SC], F32, tag="y")
                    nc.tensor.matmul(
                        wps[:, :P], ones_sb[:], ones_sb[:],
                        skip_group_check=True,
                    )

            warmup(16)

            def project_chunk(sc, loaded):
                ssl = slice(sc * SC, (sc + 1) * SC)
                xt, cos_t, sin_t = loaded
                qT_c = outqp.tile([P, H_LOC, SC], BF16, tag="qTc")

                def vproj():
                    for sti in range(NSUB):
                        st = sc * NSUB + sti
                        lsl = slice(sti * P, (sti + 1) * P)
                        psv = cyc.tile([P, H_LOC * HD], F32, tag="ps")
                        for ko in range(KO):
                            nc.tensor.matmul(
                                psv[:],
                                xt[:, ko, lsl],
                                wv_sb[:, ko],
                                start=(ko == 0),
                                stop=(ko == KO - 1),
                            )
                        if sti % 2 == 0:
                            nc.scalar.copy(v_sb[:, st, :], psv[:])
                        else:
                            nc.vector.tensor_copy(v_sb[:, st, :], psv[:])

                def qkproj():
                    for h in range(H_LOC):
                        for t in range(2):  # 0=q, 1=k
                            wcols = slice(
                                (2 * h + t) * HD, (2 * h + t + 1) * HD
                            )
                            ps = cyc.tile([P, SC], F32, tag="ps")
                            for ko in range(KO):
                                nc.tensor.matmul(
                                    ps[:],
                                    wqk_sb[:, ko, wcols],
                                    xt[:, ko],
                                    start=(ko == 0),
                                    stop=(ko == KO - 1),
                                )
                            plain = scr.tile([P, SC], F32R, tag="plain")
                            nc.scalar.copy(plain[:], ps[:])
                            rot = cyc.tile([P, SC], F32, tag="ps")
                            nc.tensor.matmul(rot[:], pt_sb[:], plain[:])
                            dst = (
                                qT_c[:, h, :] if t == 0 else kT_sb[:, h, ssl]
                            )
                            # rope: dst = plain*cos + rot*sin
                            pc = scr.tile([P, SC], F32, tag="pc")
                            nc.gpsimd.tensor_mul(pc[:], plain[:], cos_t[:])
                            tmp2 = scr.tile([P, SC], F32, tag="tmp2")
                            nc.vector.tensor_mul(tmp2[:], rot[:], sin_t[:])
                            nc.vector.tensor_add(dst, pc[:], tmp2[:])

                def take_plain(src_ps, on_dve=False, g=0):
                    plain = scr.tile(
                        [P, SC], F32R, tag=f"plain{g % 4}", name="plain"
                    )
                    if on_dve:
                        nc.vector.tensor_copy(plain[:], src_ps[:])
                    else:
                        nc.scalar.copy(plain[:], src_ps[:])
                    return plain

                def rope_rest(h, t, plain):
                    rot = cyc.tile([P, SC], F32, tag="ps")
                    nc.tensor.matmul(rot[:], pt_sb[:], plain[:])
                    dst = qT_c[:, h, :] if t == 0 else kT_sb[:, h, ssl]
                    pc = scr.tile([P, SC], F32, tag="pc")
                    nc.gpsimd.tensor_mul(pc[:], plain[:], cos_t[:])
                    tmp2 = scr.tile([P, SC], F32, tag="tmp2")
                    nc.vector.tensor_mul(tmp2[:], rot[:], sin_t[:])
                    nc.vector.tensor_add(dst, pc[:], tmp2[:])

                def rope_tail(h, t, src_ps):
                    rope_rest(h, t, take_plain(src_ps))

                def chunk0_proj():
                    # ko-major so the PE consumes (xt, wv), then wqk slices
                    # in DMA-arrival order, across borrowed idle PSUM banks
                    vbank = [
                        cyc.tile([P, H_LOC * HD], F32, tag="ps",
                                 name=f"vbank{i}")
                        for i in range(3)
                    ]
                    vbank.append(
                        ops.tile([P, SC], F32, tag="o", name="vbank3")
                    )
                    for ko in range(KO):
                        for sti in range(NSUB):
                            lsl = slice(sti * P, (sti + 1) * P)
                            nc.tensor.matmul(
                                vbank[sti][:, : H_LOC * HD],
                                xt[:, ko, lsl],
                                wv_sb[:, ko],
                                start=(ko == 0),
                                stop=(ko == KO - 1),
                                skip_group_check=True,
                            )
                    for sti in range(NSUB):
                        if sti % 2 == 0:
                            nc.scalar.copy(
                                v_sb[:, sti, :], vbank[sti][:, : H_LOC * HD]
                            )
                        else:
                            nc.vector.tensor_copy(
                                v_sb[:, sti, :], vbank[sti][:, : H_LOC * HD]
                            )
                    qbank = (
                        [cyc.tile([P, SC], F32, tag="ps", name=f"qbank{i}")
                         for i in range(3)]
                        + [ops.tile([P, SC], F32, tag="o", name=f"qbank{3+i}")
                           for i in range(2)]
                        + [yps.tile([P, SC], F32, tag="y", name=f"qbank{5+i}")
                           for i in range(2)]
                    )
                    groups = [(h, t) for h in range(H_LOC) for t in range(2)]
                    for ko in range(KO):
                        for g, (h, t) in enumerate(groups):
                            if g == 7:
                                continue  # bank-limited: group 7 runs after
                            wcols = slice(
                                (2 * h + t) * HD, (2 * h + t + 1) * HD
                            )
                            nc.tensor.matmul(
                                qbank[g][:],
                                wqk_sb[:, ko, wcols],
                                xt[:, ko],
                                start=(ko == 0),
                                stop=(ko == KO - 1),
                                skip_group_check=True,
                            )
                    # free the borrowed banks first (copies split across
                    # ACT+DVE), with group 7's group-major matmuls emitted
                    # in between so the PE stays busy during the copies
                    plains = {}
                    for g in range(2):
                        plains[g] = take_plain(qbank[g], g % 2 == 1, g)
                    h7, t7 = groups[7]
                    ps = ops.tile([P, SC], F32, tag="o", name="ps_g7")
                    wcols = slice((2 * h7 + t7) * HD, (2 * h7 + t7 + 1) * HD)
                    for ko in range(KO):
                        nc.tensor.matmul(
                            ps[:],
                            wqk_sb[:, ko, wcols],
                            xt[:, ko],
                            start=(ko == 0),
                            stop=(ko == KO - 1),
                        )
                    for g in range(2, 7):
                        plains[g] = take_plain(qbank[g], g % 2 == 1, g)
                    for g, (h, t) in enumerate(groups[:7]):
                        rope_rest(h, t, plains[g])
                    rope_tail(h7, t7, ps)

                if sc == 0:
                    # chunk 0 is fed by the startup stream in (xt, wv),
                    # then wqk order — consume in that order
                    chunk0_proj()
                else:
                    qkproj()
                    vproj()
                return qT_c

            def attend_pair(qc, qT_c, hp, outT_qc, fillers, drain_sched=None):
                """Attention for query chunk qc, heads (2hp, 2hp+1)
                interleaved per k-block (so the PE always has two
                independent softmax chains in flight), writing normalized
                outT [hd, q] slices.  `fillers` is a deque of independent
                PE-work closures drained into the pipeline's tail bubbles.
                Diagonal k-blocks are narrowed to their live q columns."""
                nkb = (qc + 1) * NSUB
                qt = {}
                o_ps = {}
                acc = {}
                for hl in range(2):
                    h = 2 * hp + hl
                    qt[hl] = qT_c[:, h, :]
                    o_ps[hl] = ops.tile([P, SC], F32, tag="o", name=f"o_ps{hl}")
                    acc[hl] = accp.tile(
                        [P, SC], F16, tag=f"acc{hl}", name=f"acc{hl}"
                    )
                stile = {}

                def q0(kb):
                    # first live q column for k-block kb (causal narrowing)
                    j = kb - qc * NSUB
                    return j * P if j > 0 else 0

                def emit_scores(kb, hl):
                    h = 2 * hp + hl
                    c0 = q0(kb)
                    t_ = cyc.tile([P, SC], F32, tag="ps")
                    nc.tensor.matmul(
                        t_[:, c0:],
                        kT_sb[:, h, kb * P : (kb + 1) * P],
                        qt[hl][:, c0:],
                        skip_group_check=True,
                    )
                    j = kb - qc * NSUB
                    if j >= 0:
                        # triangular boundary block only
                        nc.vector.tensor_add(
                            t_[:, c0 : c0 + P],
                            t_[:, c0 : c0 + P],
                            mask_sb[:, j, :],
                        )
                    stile[(kb, hl)] = t_

                if drain_sched is None:
                    drain_sched = lambda i: 1 if i % 2 == 1 else 0
                seq = [(kb, hl) for kb in range(nkb) for hl in range(2)]
                for kb, hl in seq[:LOOKAHEAD]:
                    emit_scores(kb, hl)
                for i, (kb, hl) in enumerate(seq):
                    c0 = q0(kb)
                    h = 2 * hp + hl
                    e = expp.tile([P, SC], F16, tag="e")
                    # bias -ln(16) scales e (and thus both the PV numerator
                    # and the denominator — ratio exact) by 1/16, keeping
                    # the fp16 softmax accumulator far from overflow
                    nc.scalar.activation(
                        e[:, c0:], stile.pop((kb, hl))[:, c0:], AF.Exp,
                        scale=inv_sqrt_hd, bias=expb[:],
                    )
                    nc.tensor.matmul(
                        o_ps[hl][:, c0:],
                        v_sb[:, kb, h * HD : (h + 1) * HD],
                        e[:, c0:],
                        start=(kb == 0),
                        stop=(kb == nkb - 1),
                        skip_group_check=True,
                    )
                    # softmax denominator: accumulate e on the DVE (off the
                    # critical path), partition-broadcast later via one
                    # ones-matmul per head
                    if kb == 0:
                        nc.vector.tensor_copy(acc[hl][:], e[:])
                    else:
                        nc.vector.tensor_add(
                            acc[hl][:, c0:], acc[hl][:, c0:], e[:, c0:]
                        )
                    if i + LOOKAHEAD < len(seq):
                        emit_scores(*seq[i + LOOKAHEAD])
                        for _ in range(drain_sched(i)):
                            if fillers:
                                fillers.popleft()()
                    elif fillers:
                        fillers.popleft()()
                for hl in range(2):
                    h = 2 * hp + hl
                    d_ps = cyc.tile([P, SC], F32, tag="ps")
                    nc.tensor.matmul(
                        d_ps[:], ones_sb[:], acc[hl][:],
                        skip_group_check=True,
                    )
                    recip = scr.tile([P, SC], F32, tag="recip")
                    nc.vector.reciprocal_approx_fast(recip[:], d_ps[:])
                    nc.vector.tensor_mul(
                        outT_qc[:, h, :], o_ps[hl][:], recip[:]
                    )

            def make_out_fillers(qc, outT_qc, tail=False):
                """One closure per (s-tile, d-chunk) block of the wo
                projection for query chunk qc: 4 accumulating matmuls (all
                local heads), a PSUM->SBUF bf16 copy, and the output DMA.
                Tail blocks (after the last attend) ping-pong across the
                now-idle attention PSUM banks and both copy engines."""
                work = []
                for sti in range(NSUB):
                    st = qc * NSUB + sti
                    stsl = slice(sti * P, (sti + 1) * P)
                    for dc in range(D // SC):
                        dsl = slice(dc * SC, (dc + 1) * SC)
                        bi = sti * (D // SC) + dc

                        def blk(st=st, stsl=stsl, dsl=dsl, bi=bi):
                            if tail:
                                pool, tag = [(yps, "y"), (ops, "o")][bi % 2]
                                y_ps = pool.tile([P, SC], F32, tag=tag)
                            else:
                                y_ps = yps.tile([P, SC], F32, tag="y")
                            for h in range(H_LOC):
                                nc.tensor.matmul(
                                    y_ps[:],
                                    outT_qc[:, h, stsl],
                                    wo_sb[:, h, dsl],
                                    start=(h == 0),
                                    stop=(h == H_LOC - 1),
                                )
                            y_sb = yop.tile([P, SC], BF16, tag="ysb")
                            # PSUM->bf16 staging copy alternates DVE/ACT
                            # (gpsimd cannot read PSUM) to keep the DVE,
                            # which carries the softmax accumulation, free
                            if bi % 3 != 2:
                                nc.vector.tensor_copy(y_sb[:], y_ps[:])
                            else:
                                nc.scalar.copy(y_sb[:], y_ps[:])
                            yq = (
                                nc.scalar if (tail and bi % 2 == 1)
                                else nc.sync
                            )
                            yq.dma_start(
                                y[st * P : (st + 1) * P, dsl], y_sb[:]
                            )

                        work.append(blk)
                return work

            def warm_filler():
                wps = yps.tile([P, SC], F32, tag="y")
                nc.tensor.matmul(
                    wps[:, :P], ones_sb[:], ones_sb[:],
                    skip_group_check=True,
                )

            def make_v_units(sc, xt):
                """The v projection of chunk sc as 4 self-contained filler
                units (16 accumulating matmuls + a PSUM->bf16 copy each),
                drained into the previous chunk's attend bubbles."""
                units = []
                for sti in range(NSUB):
                    st = sc * NSUB + sti
                    lsl = slice(sti * P, (sti + 1) * P)

                    def vu(st=st, lsl=lsl, sti=sti):
                        psv = yps.tile([P, SC], F32, tag="y")
                        for ko in range(KO):
                            nc.tensor.matmul(
                                psv[:],
                                xt[:, ko, lsl],
                                wv_sb[:, ko],
                                start=(ko == 0),
                                stop=(ko == KO - 1),
                                skip_group_check=True,
                            )
                        if sti % 2 == 0:
                            nc.scalar.copy(v_sb[:, st, :], psv[:])
                        else:
                            nc.vector.tensor_copy(v_sb[:, st, :], psv[:])

                    vu.kind = "v"
                    units.append(vu)
                return units

            pending = deque()
            chunks = {0: preloaded}
            for sc in range(NQC):
                if 1 <= sc < NQC - 1 and sc + 1 not in chunks:
                    # prefetch next chunk before this chunk's projection
                    chunks[sc + 1] = load_chunk(sc + 1)
                qT_c = project_chunk(sc, chunks[sc])
                if sc == 0:
                    # wo and the chunk-1 prefetch land on the scalar queue
                    # only after chunk 0's projection copies — keeps the
                    # startup stream alone on the shared DMA engines
                    for h in range(H_LOC):
                        nc.scalar.dma_start(wo_sb[:, h], wo_d[:, h])
                    chunks[1] = load_chunk(1)
                if sc == 0:
                    # qc0 has no wo fillers yet: drain clock-gate warmers
                    # into its softmax-wait bubbles instead
                    pending.extend([warm_filler] * 24)
                outT_qc = outqp.tile([P, H_LOC, SC], BF16, tag="outq")
                ds = (lambda i: 2) if sc == 0 else None
                for hp in range(2):
                    attend_pair(sc, qT_c, hp, outT_qc, pending, ds)
                pending = deque(f for f in pending if f is not warm_filler)
                pending.extend(
                    make_out_fillers(sc, outT_qc, tail=(sc == NQC - 1))
                )
            while pending:
                pending.popleft()()

    nc.compile()
    return nc


_NC_CACHE = {}


def _get_nc():
    if "nc" not in _NC_CACHE:
        _NC_CACHE["nc"] = _build_core_kernel()
    return _NC_CACHE["nc"]


def _rope_perm_T() -> np.ndarray:
    # rotate_half as a matrix: (P_rh @ q)[d] = -q[d+HD/2] for d < HD/2,
    # q[d-HD/2] otherwise.  Returns P_rh.T for use as matmul lhsT.
    P_rh = np.zeros((HD, HD), dtype=np.float32)
    half = HD // 2
    for i in range(half):
        P_rh[i, half + i] = -1.0
        P_rh[half + i, i] = 1.0
    return np.ascontiguousarray(P_rh.T)


def _is_causal(m: np.ndarray) -> bool:
    tril = np.tril(np.ones((S, S), dtype=bool))
    if not np.all(m[tril] == 0.0):
        return False
    upper = m[~tril]
    return bool(upper.size == 0 or np.all(upper <= -1.0e8))


def _bf16(x: np.ndarray) -> np.ndarray:
    return np.ascontiguousarray(x).astype(ml_dtypes.bfloat16)


def _reference_numpy(x, cos, sin, mask, wq, wk, wv, wo):
    # generic-mask fallback (never hit for the causal reference mask)
    def rot_half(t):
        t1, t2 = np.split(t, 2, axis=-1)
        return np.concatenate((-t2, t1), axis=-1)

    H = N_HEADS
    q = (x @ wq.T).reshape(B, S, H, HD).transpose(0, 2, 1, 3)
    k = (x @ wk.T).reshape(B, S, H, HD).transpose(0, 2, 1, 3)
    v = (x @ wv.T).reshape(B, S, H, HD).transpose(0, 2, 1, 3)
    c = cos[None, None]
    s = sin[None, None]
    q = q * c + rot_half(q) * s
    k = k * c + rot_half(k) * s
    scores = np.einsum("bhqd,bhkd->bhqk", q, k) / np.sqrt(np.float32(HD))
    scores = scores + mask
    scores -= scores.max(axis=-1, keepdims=True)
    p = np.exp(scores)
    p /= p.sum(axis=-1, keepdims=True)
    out = np.einsum("bhqk,bhkd->bhqd", p, v)
    out = out.transpose(0, 2, 1, 3).reshape(B, S, D)
    return (out @ wo.T).astype(np.float32)


# module-level: results of the last traced run (for test harnesses)
last_exec_time_ns = None
last_profile_json = None


def kernel(x, cos, sin, mask, wq, wk, wv, wo, _trace=False):
    x = np.asarray(x, dtype=np.float32)
    cos = np.asarray(cos, dtype=np.float32)
    sin = np.asarray(sin, dtype=np.float32)
    mask = np.asarray(mask, dtype=np.float32)
    wq = np.asarray(wq, dtype=np.float32)
    wk = np.asarray(wk, dtype=np.float32)
    wv = np.asarray(wv, dtype=np.float32)
    wo = np.asarray(wo, dtype=np.float32)

    m2d = mask.reshape(S, S)
    if not _is_causal(m2d):
        return _reference_numpy(x, cos, sin, mask, wq, wk, wv, wo)
    nc = _get_nc()

    scale = np.float32(np.sqrt(HD))
    # [k, q] transposed causal boundary blocks: mask_h[ki, j, q_local]
    mt = np.ascontiguousarray((m2d[:SC, :SC] * scale).T).reshape(NSUB, P, NSUB, P)
    mask_h = np.ascontiguousarray(
        np.stack([mt[j, :, j, :] for j in range(NSUB)], axis=1)
    )
    # cos/sin chunk-tiled: cs[sc, {cos,sin}, hd, s_local]
    cs = np.stack([cos.T, sin.T], axis=0).reshape(2, HD, NQC, SC)
    cs = np.ascontiguousarray(cs.transpose(2, 0, 1, 3), dtype=np.float32)
    ptT = _rope_perm_T()
    ones = np.ones((P, P), dtype=np.float32)

    # x chunk-tiled: xt[sc, ki, ko, s_local]
    xts = []
    for b in range(B):
        xT = x[b].T.reshape(KO, P, NQC, SC)
        xts.append(_bf16(xT.transpose(2, 1, 0, 3)))

    in_maps = []
    for c in range(N_CORES):
        b = c // (N_CORES // B)
        hg = c % (N_CORES // B)
        # qkv packed per head: [q_h0|k_h0|...|q_h3|k_h3|v_h0..v_h3],
        # laid out [ko, ki, col]
        cols = []
        for h in range(H_LOC):
            hh = hg * H_LOC + h
            cols.append(wq[hh * HD : (hh + 1) * HD].T)
            cols.append(wk[hh * HD : (hh + 1) * HD].T)
        for h in range(H_LOC):
            hh = hg * H_LOC + h
            cols.append(wv[hh * HD : (hh + 1) * HD].T)
        wpack = np.concatenate(cols, axis=1)  # [D, 1536]
        wpack = wpack.reshape(KO, P, QKV_W)
        wqk_h = np.ascontiguousarray(wpack[:, :, : 2 * H_LOC * HD])
        wv_h = np.ascontiguousarray(
            wpack[:, :, 2 * H_LOC * HD :].transpose(1, 0, 2)
        )
        # wo rows for this head group, laid out [ki, h, d]
        rows = slice(hg * H_LOC * HD, (hg + 1) * H_LOC * HD)
        wot = wo[:, rows].T.reshape(H_LOC, P, D)
        wot = np.ascontiguousarray(wot.transpose(1, 0, 2))
        in_maps.append(
            {
                "xt": xts[b],
                "wqk": _bf16(wqk_h),
                "wv": _bf16(wv_h),
                "wo": _bf16(wot),
                "cs": cs,
                "pt": ptT,
                "ones": _bf16(ones),
                "mask": mask_h,
            }
        )

    kw = {}
    if _trace:
        kw = dict(trace=True)
    res = run_bass_kernel_spmd(
        nc, in_maps, core_ids=list(range(N_CORES)), **kw
    )
    global last_exec_time_ns, last_profile_json
    last_exec_time_ns = res.exec_time_ns
    last_profile_json = res.profile_json

    out = np.empty((B, S, D), dtype=np.float32)
    gs = N_CORES // B
    for b in range(B):
        acc = res.results[b * gs]["y"].astype(np.float32)
        for g in range(1, gs):
            acc += res.results[b * gs + g]["y"].astype(np.float32)
        out[b] = acc
    return out
